# revision 1
# baseline (speedup 1.0000x reference)
"""Trainium2 Bass kernel for nn_DAAdj_57114475102829 (GAT-style message passing).

Math (N=4096, F=256, H=8):
  s = x @ Ws.T            [N, H]   (Ws = W_dist[:, :F])
  t'= x @ Wt.T + b_dist   [N, H]   (Wt = W_dist[:, F:])
  z[i,j,h] = s[i,h] + t'[j,h] + (i==j)*selfbias[h]
  heads = softmax(relu(z), axis=j)
  out[i,j] = sum_h heads[i,j,h]*W_merge[0,h] + b_merge[0]

Sharding: rows of i across 8 cores (512 rows each); softmax over j is local.
Each core receives x ROTATED by -c*512 rows so that every core's program is
identical (its own rows are rows 0..512 of its input; the diagonal lands at
j==i in rotated coordinates). The host unrotates output columns.

Per-core device pipeline (per row-block b of 128, per head h):
  E0 = exp(T[j,h] + s[i,h])          scalar engine (bias=per-partition s)
  E0[:, diag] *= exp(selfbias[h])    vector (128x128 only)  [exp(relu(z))=max(exp(z),1)]
  EH = max(E0, 1), denom=sum_j EH    vector tensor_scalar with accum_out
  c = W_merge[h]/denom               vector (per-partition)
  PSUM[:, jc] += diag(c) @ EH_chunk  tensor engine (fp32r), accumulate over h
  out = PSUM + b_merge  -> DMA       vector copy + HWDGE DMA
"""
import sys

sys.path.insert(0, "/opt/trn_rl_repo")

import numpy as np
import concourse.bacc as bacc
from concourse import mybir
from concourse.tile import TileContext
from concourse.bass_utils import run_bass_kernel_spmd

N, F, H = 4096, 256, 8
NCORES = 8
ROWS = N // NCORES  # 512 rows per core
P = 128
NB = ROWS // P      # 4 row blocks per core
JC = 512            # merge chunk = 1 PSUM bank of fp32
NJC = N // JC       # 8 chunks
FP32 = mybir.dt.float32
FP32R = mybir.dt.float32r
AL = mybir.AluOpType
AF = mybir.ActivationFunctionType

# Merge-matmul dtype: fp32 is exact (2-pass hi/lo on the PE); fp32r is fast
# but numerically broken on TRN2 hardware.
MERGE_DT = FP32

_CACHE = {}


def _build():
    nc = bacc.Bacc("TRN2", target_bir_lowering=False, debug=False, num_devices=NCORES)

    x_d = nc.dram_tensor("x", [N, F], FP32, kind="ExternalInput")
    wd_d = nc.dram_tensor("wd", [H, 2 * F], FP32, kind="ExternalInput")
    bd_d = nc.dram_tensor("bd", [H, 1], FP32, kind="ExternalInput")
    wm_d = nc.dram_tensor("wm", [1, H], FP32, kind="ExternalInput")
    bm_d = nc.dram_tensor("bm", [1, 1], FP32, kind="ExternalInput")
    sb_d = nc.dram_tensor("sb", [1, H], FP32, kind="ExternalInput")
    out_d = nc.dram_tensor("out", [ROWS, N], FP32, kind="ExternalOutput")

    with TileContext(nc) as tc:
        with tc.tile_pool(name="persist", bufs=1) as persist:
            # ---- persistent tiles ----
            t_all = persist.tile([P, H, N], FP32, tag="t_all")       # 128 KiB/part
            mask = persist.tile([P, P], FP32, tag="mask")            # identity
            exp_i = persist.tile([P, H, P], FP32, tag="expi")        # diag exp(sb)
            s_all = persist.tile([P, NB, H], FP32, tag="s_all")
            wm_b = persist.tile([P, H], FP32, tag="wm_b")
            bm_c = persist.tile([P, 1], FP32, tag="bm_c")
            sb_b = persist.tile([P, H], FP32, tag="sb_b")
            bd_c = persist.tile([H, 1], FP32, tag="bd_c")
            it_p = persist.tile([P, 1], FP32, tag="it_p")
            it_f = persist.tile([P, P], FP32, tag="it_f")

            with tc.tile_pool(name="dram", bufs=1, space="DRAM") as dpool:
                tpd = dpool.tile([H, N], FP32)

                # ================= startup =================
                # broadcast small params across partitions (stride-0 DMA)
                nc.sync.dma_start(out=wm_b, in_=wm_d[0:1, :].to_broadcast((P, H)))
                nc.sync.dma_start(out=bm_c, in_=bm_d[0:1, :].to_broadcast((P, 1)))
                nc.sync.dma_start(out=sb_b, in_=sb_d[0:1, :].to_broadcast((P, H)))
                nc.sync.dma_start(out=bd_c, in_=bd_d[:, :])

                # identity mask via iota + compare
                nc.gpsimd.iota(
                    it_p, [[0, 1]], channel_multiplier=1,
                    allow_small_or_imprecise_dtypes=True,
                )
                nc.gpsimd.iota(
                    it_f, [[1, P]], channel_multiplier=0,
                    allow_small_or_imprecise_dtypes=True,
                )
                nc.vector.tensor_scalar(mask, it_f, it_p[:, 0:1], None, AL.is_equal)

                # exp_i[h] = 1 + mask*(exp(sb[h]) - 1)
                esb = persist.tile([P, H], FP32, tag="esb")
                nc.scalar.activation(esb, sb_b, AF.Exp)
                nc.vector.tensor_scalar(esb, esb, -1.0, None, AL.add)
                for h in range(H):
                    nc.vector.tensor_scalar(
                        exp_i[:, h, :], mask, esb[:, h : h + 1], 1.0, AL.mult, AL.add
                    )

                with (
                    tc.tile_pool(name="su1", bufs=1) as su1,
                    tc.tile_pool(name="su", bufs=2) as su,
                ):
                    # W transposes via strided DMA (tiny):
                    # wst[fh] = Ws.T[fh*128:(fh+1)*128, :], wtt likewise for Wt
                    wst = []
                    wtt = []
                    for fh in range(2):
                        wsts = su.tile([P, H], FP32, tag=f"wst{fh}")
                        nc.sync.dma_start(
                            out=wsts,
                            in_=wd_d[0:H, fh * P : (fh + 1) * P].transpose([1, 0]),
                        )
                        wst.append(wsts)
                        wtts = su.tile([P, H], FP32, tag=f"wtt{fh}")
                        nc.sync.dma_start(
                            out=wtts,
                            in_=wd_d[0:H, F + fh * P : F + (fh + 1) * P].transpose(
                                [1, 0]
                            ),
                        )
                        wtt.append(wtts)

                    # x.T [256, 4096] as two [128, 4096] tiles, via PE transposes
                    xt = [
                        su1.tile([P, N], FP32, tag=f"xt{fh}", name=f"xt{fh}")
                        for fh in range(2)
                    ]
                    with tc.tile_pool(name="ps_tr", bufs=4, space="PSUM") as ps_tr:
                        for rt in range(N // ROWS):  # 8 groups of 4 row-tiles
                            xbig = su.tile([P, NB, F], FP32, tag="xbig")
                            nc.sync.dma_start(
                                out=xbig,
                                in_=x_d[rt * ROWS : (rt + 1) * ROWS, :].rearrange(
                                    "(a p) f -> p a f", p=P
                                ),
                            )
                            for a in range(NB):
                                col = rt * ROWS + a * P
                                for fh in range(2):
                                    pst = ps_tr.tile([P, P], FP32, tag="tr")
                                    nc.tensor.transpose(
                                        pst, xbig[:, a, fh * P : (fh + 1) * P], mask
                                    )
                                    eng = nc.vector if (a + fh) % 2 == 0 else nc.scalar
                                    if eng is nc.vector:
                                        nc.vector.tensor_copy(
                                            xt[fh][:, col : col + P], pst
                                        )
                                    else:
                                        nc.scalar.copy(xt[fh][:, col : col + P], pst)

                    with tc.tile_pool(name="ps_stp", bufs=2, space="PSUM") as ps_stp:
                        # s_all[i, b, h] for this core's rows (= cols 0..512 of x.T)
                        for b in range(NB):
                            ps_s = ps_stp.tile([P, H], FP32, tag="s")
                            nc.tensor.matmul(
                                ps_s,
                                lhsT=xt[0][:, b * P : (b + 1) * P],
                                rhs=wst[0],
                                start=True,
                                stop=False,
                            )
                            nc.tensor.matmul(
                                ps_s,
                                lhsT=xt[1][:, b * P : (b + 1) * P],
                                rhs=wst[1],
                                start=False,
                                stop=True,
                            )
                            nc.vector.tensor_copy(s_all[:, b, :], ps_s)

                        # t'_T [8, 4096] = Wt @ x.T + b_dist
                        tp_t = su1.tile([H, N], FP32, tag="tp_t")
                        for jc in range(NJC):
                            ps_t = ps_stp.tile([H, JC], FP32, tag="t")
                            nc.tensor.matmul(
                                ps_t,
                                lhsT=wtt[0],
                                rhs=xt[0][:, jc * JC : (jc + 1) * JC],
                                start=True,
                                stop=False,
                            )
                            nc.tensor.matmul(
                                ps_t,
                                lhsT=wtt[1],
                                rhs=xt[1][:, jc * JC : (jc + 1) * JC],
                                start=False,
                                stop=True,
                            )
                            nc.scalar.activation(
                                tp_t[:, jc * JC : (jc + 1) * JC],
                                ps_t,
                                AF.Identity,
                                bias=bd_c[:, 0:1],
                            )

                        # round-trip t' through HBM, broadcast to 128 partitions
                        nc.sync.dma_start(out=tpd, in_=tp_t)
                        for h in range(H):
                            nc.sync.dma_start(
                                out=t_all[:, h, :],
                                in_=tpd[h : h + 1, :].to_broadcast((P, N)),
                            )

                # ================= steady state =================
                with (
                    tc.tile_pool(name="big", bufs=3) as big,
                    tc.tile_pool(name="dcp", bufs=2) as dcp,
                    tc.tile_pool(name="small", bufs=6) as small,
                    tc.tile_pool(name="ost", bufs=2) as ost,
                    tc.tile_pool(name="mps", bufs=1, space="PSUM") as mps,
                ):
                    for b in range(NB):
                        dr = b * P  # diagonal column range start
                        psum_tiles = [
                            mps.tile([P, JC], FP32, tag=f"m{jc}", name=f"m{b}_{jc}")
                            for jc in range(NJC)
                        ]
                        for h in range(H):
                            e0 = big.tile([P, N], FP32, tag="big")
                            nc.scalar.activation(
                                e0,
                                t_all[:, h, :],
                                AF.Exp,
                                bias=s_all[:, b, h : h + 1],
                            )
                            # diagonal selfbias fix (only i==j block columns)
                            nc.vector.tensor_tensor(
                                out=e0[:, dr : dr + P],
                                in0=e0[:, dr : dr + P],
                                in1=exp_i[:, h, :],
                                op=AL.mult,
                            )
                            eh = big.tile([P, N], MERGE_DT, tag="big")
                            denom = small.tile([P, 1], FP32, tag="denom")
                            nc.vector.tensor_scalar(
                                eh, e0, 1.0, None, AL.max, AL.add, accum_out=denom
                            )
                            recip = small.tile([P, 1], FP32, tag="recip")
                            nc.vector.reciprocal(recip, denom)
                            cvec = small.tile([P, 1], FP32, tag="cvec")
                            nc.vector.tensor_scalar(
                                cvec, recip, wm_b[:, h : h + 1], None, AL.mult
                            )
                            dc = dcp.tile([P, P], MERGE_DT, tag="dc")
                            nc.vector.tensor_scalar(
                                dc, mask, cvec[:, 0:1], None, AL.mult
                            )
                            for jc in range(NJC):
                                nc.tensor.matmul(
                                    psum_tiles[jc],
                                    lhsT=dc,
                                    rhs=eh[:, jc * JC : (jc + 1) * JC],
                                    start=(h == 0),
                                    stop=(h == H - 1),
                                )
                        # drain block: PSUM -> SBUF (+b_merge) -> HBM
                        for jh in range(2):
                            o = ost.tile([P, N // 2], FP32, tag="ost")
                            for q in range(NJC // 2):
                                jc = jh * (NJC // 2) + q
                                nc.vector.tensor_scalar(
                                    o[:, q * JC : (q + 1) * JC],
                                    psum_tiles[jc],
                                    bm_c[:, 0:1],
                                    None,
                                    AL.add,
                                )
                            nc.sync.dma_start(
                                out=out_d[
                                    b * P : (b + 1) * P,
                                    jh * (N // 2) : (jh + 1) * (N // 2),
                                ],
                                in_=o,
                            )

    nc.compile()
    return nc


def _get_nc():
    if "nc" not in _CACHE:
        _CACHE["nc"] = _build()
    return _CACHE["nc"]


def _in_maps(inputs):
    x = np.ascontiguousarray(np.asarray(inputs["x"], dtype=np.float32))
    W_dist = np.ascontiguousarray(np.asarray(inputs["W_dist"], dtype=np.float32))
    b_dist = np.asarray(inputs["b_dist"], dtype=np.float32).reshape(H, 1)
    W_merge = np.asarray(inputs["W_merge"], dtype=np.float32).reshape(1, H)
    b_merge = np.asarray(inputs["b_merge"], dtype=np.float32).reshape(1, 1)
    selfbias = np.asarray(inputs["selfbias"], dtype=np.float32).reshape(1, H)
    in_maps = []
    for c in range(NCORES):
        in_maps.append(
            {
                "x": np.ascontiguousarray(np.roll(x, -c * ROWS, axis=0)),
                "wd": W_dist,
                "bd": b_dist,
                "wm": W_merge,
                "bm": b_merge,
                "sb": selfbias,
            }
        )
    return in_maps


def _assemble(results):
    out = np.empty((N, N), dtype=np.float32)
    for c in range(NCORES):
        out[c * ROWS : (c + 1) * ROWS, :] = np.roll(
            results[c]["out"], c * ROWS, axis=1
        )
    return out


def kernel(x, W_dist, b_dist, W_merge, b_merge, selfbias):
    nc = _get_nc()
    in_maps = _in_maps(
        {
            "x": x,
            "W_dist": W_dist,
            "b_dist": b_dist,
            "W_merge": W_merge,
            "b_merge": b_merge,
            "selfbias": selfbias,
        }
    )
    res = run_bass_kernel_spmd(nc, in_maps, core_ids=list(range(NCORES)))
    return _assemble(res.results)



# revision 8
# speedup vs baseline: 1.5891x; 1.5891x over previous
"""Trainium2 Bass kernel for nn_DAAdj_57114475102829 (GAT-style message passing).

Math (N=4096, F=256, H=8):
  s = x @ Ws.T             [N, H]   (Ws = W_dist[:, :F])
  t'= x @ Wt.T + b_dist    [N, H]   (Wt = W_dist[:, F:])
  z[i,j,h] = s[i,h] + t'[j,h] + (i==j)*selfbias[h]
  heads = softmax(relu(z), axis=j)
  out[i,j] = sum_h heads[i,j,h]*W_merge[0,h] + b_merge[0]

Key identities:
  exp(relu(z)) = max(exp(z), 1)   and   exp(z) = a_i * u_j  with
  a = exp(s), u = exp(t').  Define r = max(u_j, 1/a_i)  =>  numer = a*r,
  Z = a * sum_j r, so heads = r / sum_j r and the exp/a cancels entirely:
  out[i,j] = sum_h C_ih * r_ijh + b_merge,  C = W_merge[h] / sum_j r.

Layout: rows i sharded across 8 cores (x rotated so each core's rows are
0..512 local; diag at j==i).  Per core, i is processed in 16 groups of 32.
r is materialized in bf16 as [128=(h4,q32), 4096=j] tiles (two h-halves),
produced by ONE DVE tensor_scalar (op0=max) whose accum_out gives sum_j r.
The h-sum 'merge' runs on the PE as block-diag stationary matmuls
(k=(h,q)=128, m=32) with 4-way column tiling so 4 groups stream
concurrently; C is baked into the stationary.
"""
import sys

sys.path.insert(0, "/opt/trn_rl_repo")

import numpy as np
import ml_dtypes
import concourse.bacc as bacc
from concourse import mybir
from concourse.tile import TileContext
from concourse.bass_utils import run_bass_kernel_spmd

N, F, H = 4096, 256, 8
NCORES = 8
ROWS = N // NCORES   # 512 rows per core
P = 128
NG = ROWS // 32      # 16 groups of 32 rows
NSB = ROWS // P      # 4 superblocks of 128 rows
JC = 512             # merge chunk = 1 PSUM bank of fp32
NJC = N // JC        # 8 chunks
FP32 = mybir.dt.float32
BF16 = mybir.dt.bfloat16
AL = mybir.AluOpType
AF = mybir.ActivationFunctionType

# m-production form: "ts_accum" (tensor_scalar + accum_out) or "stt"
# (scalar_tensor_tensor with a zeros tile).  Chosen by HW probe.
MPROD = "ts_accum"

_CACHE = {}


def _build():
    nc = bacc.Bacc("TRN2", target_bir_lowering=False, debug=False, num_devices=NCORES)

    x_d = nc.dram_tensor("x", [N, F], BF16, kind="ExternalInput")
    wst_d = nc.dram_tensor("wst", [F, H], BF16, kind="ExternalInput")   # Ws.T
    wtt_d = nc.dram_tensor("wtt", [F, H], BF16, kind="ExternalInput")   # Wt.T
    bd_d = nc.dram_tensor("bd", [H, 1], FP32, kind="ExternalInput")
    bm_d = nc.dram_tensor("bm", [1, 1], FP32, kind="ExternalInput")
    m32_d = nc.dram_tensor("m32", [P, 32], BF16, kind="ExternalInput")   # delta(q,m)
    mwA_d = nc.dram_tensor("mwA", [P, 32], BF16, kind="ExternalInput")   # Wm[h]*delta
    mwB_d = nc.dram_tensor("mwB", [P, 32], BF16, kind="ExternalInput")
    evA_d = nc.dram_tensor("evA", [P, 1], FP32, kind="ExternalInput")    # exp(sb[h])
    evB_d = nc.dram_tensor("evB", [P, 1], FP32, kind="ExternalInput")
    id_d = nc.dram_tensor("idm", [P, P], BF16, kind="ExternalInput")     # identity
    out_d = nc.dram_tensor("out", [ROWS, N], FP32, kind="ExternalOutput")

    with TileContext(nc) as tc:
        with tc.tile_pool(name="persist", bufs=1) as persist:
            u_rep = [persist.tile([P, N], BF16, tag=f"urep{hf}", name=f"urep{hf}") for hf in range(2)]
            mask32 = persist.tile([P, 32], BF16, tag="m32")
            maskwm = [persist.tile([P, 32], BF16, tag=f"mw{hf}", name=f"mw{hf}") for hf in range(2)]
            ev = [persist.tile([P, 1], FP32, tag=f"ev{hf}", name=f"ev{hf}") for hf in range(2)]
            idm = persist.tile([P, P], BF16, tag="idm")
            bm_c = persist.tile([P, 1], FP32, tag="bm")
            bd_c = persist.tile([H, 1], FP32, tag="bd")
            inva = [persist.tile([P, NG], FP32, tag=f"inva{hf}", name=f"inva{hf}") for hf in range(2)]
            zr = [persist.tile([P, NG], FP32, tag=f"zr{hf}", name=f"zr{hf}") for hf in range(2)]
            dvec = [persist.tile([P, NG], FP32, tag=f"dvec{hf}", name=f"dvec{hf}") for hf in range(2)]

            nc.sync.dma_start(out=mask32, in_=m32_d[:, :])
            nc.sync.dma_start(out=maskwm[0], in_=mwA_d[:, :])
            nc.sync.dma_start(out=maskwm[1], in_=mwB_d[:, :])
            nc.sync.dma_start(out=ev[0], in_=evA_d[:, :])
            nc.sync.dma_start(out=ev[1], in_=evB_d[:, :])
            nc.sync.dma_start(out=idm, in_=id_d[:, :])
            nc.sync.dma_start(out=bm_c, in_=bm_d[0:1, :].to_broadcast((P, 1)))
            nc.sync.dma_start(out=bd_c, in_=bd_d[:, :])

            with tc.tile_pool(name="dram", bufs=1, space="DRAM") as dpool:
                u_hbm = dpool.tile([H, N], BF16)
                # swizzled layouts [g, (h q)] so the gather back to SBUF is a
                # plain 2-d transpose AP (partition=(h q), free=g)
                inva_hbm = dpool.tile([NG, 2 * P], FP32)
                uown_hbm = dpool.tile([NG, 2 * P], BF16)

                # ============ startup: x^T, t', u, s, inva ============
                with (
                    tc.tile_pool(name="su1", bufs=1) as su1,
                    tc.tile_pool(name="su", bufs=3) as su,
                    tc.tile_pool(name="ps_tr", bufs=4, space="PSUM") as ps_tr,
                    tc.tile_pool(name="ps_t", bufs=2, space="PSUM") as ps_t,
                ):
                    wst = [su1.tile([P, H], BF16, tag=f"wst{fh}", name=f"wst{fh}") for fh in range(2)]
                    wtt = [su1.tile([P, H], BF16, tag=f"wtt{fh}", name=f"wtt{fh}") for fh in range(2)]
                    for fh in range(2):
                        nc.sync.dma_start(
                            out=wst[fh], in_=wst_d[fh * P : (fh + 1) * P, :]
                        )
                        nc.sync.dma_start(
                            out=wtt[fh], in_=wtt_d[fh * P : (fh + 1) * P, :]
                        )

                    xt = [su1.tile([P, N], BF16, tag=f"xt{fh}", name=f"xt{fh}") for fh in range(2)]
                    for rt in range(N // ROWS):  # 8 chunks of 512 rows
                        xbig = su.tile([P, NSB, F], BF16, tag="xbig")
                        nc.sync.dma_start(
                            out=xbig,
                            in_=x_d[rt * ROWS : (rt + 1) * ROWS, :].rearrange(
                                "(a p) f -> p a f", p=P
                            ),
                        )
                        for a in range(NSB):
                            col = rt * ROWS + a * P
                            for fh in range(2):
                                pst = ps_tr.tile([P, P], BF16, tag="tr")
                                nc.tensor.transpose(
                                    pst, xbig[:, a, fh * P : (fh + 1) * P], idm
                                )
                                if (a + fh) % 2 == 0:
                                    nc.vector.tensor_copy(
                                        xt[fh][:, col : col + P], pst
                                    )
                                else:
                                    nc.scalar.copy(xt[fh][:, col : col + P], pst)

                        # t' -> u for this 512-col chunk
                        cols = slice(rt * ROWS, (rt + 1) * ROWS)
                        ps_tc = ps_t.tile([H, ROWS], FP32, tag="t")
                        nc.tensor.matmul(
                            ps_tc, lhsT=wtt[0], rhs=xt[0][:, cols],
                            start=True, stop=False,
                        )
                        nc.tensor.matmul(
                            ps_tc, lhsT=wtt[1], rhs=xt[1][:, cols],
                            start=False, stop=True,
                        )
                        u_sb = su.tile([H, ROWS], BF16, tag="usb")
                        nc.scalar.activation(
                            u_sb, ps_tc, AF.Exp, bias=bd_c[:, 0:1]
                        )
                        nc.sync.dma_start(out=u_hbm[:, cols], in_=u_sb)
                        if rt == 0:
                            # own-row u values, swizzled [g, (h q)]
                            nc.sync.dma_start(
                                out=uown_hbm[:, :].rearrange(
                                    "g (h q) -> h g q", h=H
                                ),
                                in_=u_sb.rearrange("h (g q) -> h g q", q=32),
                            )
                        for hf in range(2):
                            for h in range(4):
                                nc.sync.dma_start(
                                    out=u_rep[hf][32 * h : 32 * h + 32, cols],
                                    in_=u_hbm[
                                        4 * hf + h : 4 * hf + h + 1, cols
                                    ].to_broadcast((32, ROWS)),
                                )

                        if rt == 0:
                            ps_s = ps_t.tile([H, ROWS], FP32, tag="s")
                            nc.tensor.matmul(
                                ps_s, lhsT=wst[0], rhs=xt[0][:, 0:ROWS],
                                start=True, stop=False,
                            )
                            nc.tensor.matmul(
                                ps_s, lhsT=wst[1], rhs=xt[1][:, 0:ROWS],
                                start=False, stop=True,
                            )
                            inva_sb = su.tile([H, ROWS], FP32, tag="isb")
                            nc.scalar.activation(
                                inva_sb, ps_s, AF.Exp, scale=-1.0
                            )
                            nc.sync.dma_start(
                                out=inva_hbm[:, :].rearrange(
                                    "g (h q) -> h g q", h=H
                                ),
                                in_=inva_sb.rearrange("h (g q) -> h g q", q=32),
                            )
                            for hf in range(2):
                                nc.sync.dma_start(
                                    out=inva[hf],
                                    in_=inva_hbm[
                                        :, hf * P : (hf + 1) * P
                                    ].rearrange("g p -> p g"),
                                )

                    # diag prep: u at own rows, Delta_r per (k, group)
                    uown_b = [su1.tile([P, NG], BF16, tag=f"uob{hf}", name=f"uob{hf}") for hf in range(2)]
                    for hf in range(2):
                        nc.sync.dma_start(
                            out=uown_b[hf],
                            in_=uown_hbm[:, hf * P : (hf + 1) * P].rearrange(
                                "g p -> p g"
                            ),
                        )
                    for hf in range(2):
                        uE = su1.tile([P, NG], FP32, tag=f"uE{hf}")
                        m2 = su1.tile([P, NG], FP32, tag=f"m2{hf}")
                        m1 = su1.tile([P, NG], FP32, tag=f"m1{hf}")
                        nc.vector.tensor_scalar(
                            uE, uown_b[hf], ev[hf][:, 0:1], None, AL.mult
                        )
                        nc.vector.tensor_tensor(
                            out=m2, in0=uE, in1=inva[hf], op=AL.max
                        )
                        nc.vector.tensor_tensor(
                            out=m1, in0=uown_b[hf], in1=inva[hf], op=AL.max
                        )
                        nc.vector.tensor_tensor(
                            out=dvec[hf], in0=m2, in1=m1, op=AL.subtract
                        )

                # ============ steady state ============
                with (
                    tc.tile_pool(name="mtl", bufs=12) as mtl,
                    tc.tile_pool(name="small", bufs=8) as small,
                    tc.tile_pool(name="ost", bufs=3) as ost,
                    tc.tile_pool(name="mps", bufs=4, space="PSUM") as mps,
                ):
                    zeros = None
                    if MPROD == "stt":
                        zeros = persist.tile([P, N], BF16, tag="zeros")
                        nc.vector.memset(zeros, 0.0)

                    for S in range(NSB):
                        r_t = {}
                        for cg in range(4):
                            g = 4 * S + cg
                            for hf in range(2):
                                rt_ = mtl.tile([P, N], BF16, tag="m")
                                if MPROD == "stt":
                                    nc.vector.scalar_tensor_tensor(
                                        rt_, u_rep[hf], inva[hf][:, g : g + 1],
                                        zeros, AL.max, AL.add,
                                        accum_out=zr[hf][:, g : g + 1],
                                    )
                                else:
                                    nc.vector.tensor_scalar(
                                        rt_, u_rep[hf], inva[hf][:, g : g + 1],
                                        None, AL.max, AL.add,
                                        accum_out=zr[hf][:, g : g + 1],
                                    )
                                # diagonal selfbias fix on cols [32g, 32g+32)
                                fix = small.tile([P, 32], BF16, tag="fix")
                                nc.vector.tensor_scalar(
                                    fix, mask32, dvec[hf][:, g : g + 1], None,
                                    AL.mult,
                                )
                                nc.vector.tensor_tensor(
                                    out=rt_[:, 32 * g : 32 * g + 32],
                                    in0=rt_[:, 32 * g : 32 * g + 32],
                                    in1=fix, op=AL.add,
                                )
                                r_t[(cg, hf)] = rt_

                        dc = {}
                        for hf in range(2):
                            zt = small.tile([P, 4], FP32, tag="zt")
                            nc.vector.tensor_tensor(
                                out=zt, in0=zr[hf][:, 4 * S : 4 * S + 4],
                                in1=dvec[hf][:, 4 * S : 4 * S + 4], op=AL.add,
                            )
                            rc = small.tile([P, 4], FP32, tag="rc")
                            nc.vector.reciprocal(rc, zt)
                            for cg in range(4):
                                d = small.tile([P, 32], BF16, tag="dc")
                                nc.vector.tensor_scalar(
                                    d, maskwm[hf], rc[:, cg : cg + 1], None,
                                    AL.mult,
                                )
                                dc[(cg, hf)] = d

                        for jh in range(2):
                            o = ost.tile([P, N // 2], FP32, tag="o")
                            for q in range(NJC // 2):
                                jc = jh * (NJC // 2) + q
                                pt = mps.tile([P, JC], FP32, tag="pt")
                                for cg in range(4):
                                    nc.tensor.matmul(
                                        pt[32 * cg : 32 * cg + 32, :],
                                        lhsT=dc[(cg, 0)],
                                        rhs=r_t[(cg, 0)][
                                            :, jc * JC : (jc + 1) * JC
                                        ],
                                        start=True, stop=False,
                                        tile_position=(0, 32 * cg),
                                    )
                                    nc.tensor.matmul(
                                        pt[32 * cg : 32 * cg + 32, :],
                                        lhsT=dc[(cg, 1)],
                                        rhs=r_t[(cg, 1)][
                                            :, jc * JC : (jc + 1) * JC
                                        ],
                                        start=False, stop=True,
                                        tile_position=(0, 32 * cg),
                                    )
                                nc.scalar.activation(
                                    o[:, q * JC : (q + 1) * JC], pt,
                                    AF.Identity, bias=bm_c[:, 0:1],
                                )
                            nc.sync.dma_start(
                                out=out_d[
                                    S * P : (S + 1) * P,
                                    jh * (N // 2) : (jh + 1) * (N // 2),
                                ],
                                in_=o,
                            )

    nc.compile()
    return nc


def _get_nc():
    if "nc" not in _CACHE:
        _CACHE["nc"] = _build()
    return _CACHE["nc"]


def _host_consts(W_merge, selfbias):
    q = np.arange(P) % 32
    h = np.arange(P) // 32
    m32 = (q[:, None] == np.arange(32)[None, :]).astype(ml_dtypes.bfloat16)
    wm = W_merge.reshape(H)
    mwA = (wm[h][:, None] * (q[:, None] == np.arange(32)[None, :])).astype(
        ml_dtypes.bfloat16
    )
    mwB = (wm[4 + h][:, None] * (q[:, None] == np.arange(32)[None, :])).astype(
        ml_dtypes.bfloat16
    )
    esb = np.exp(selfbias.reshape(H).astype(np.float64)).astype(np.float32)
    evA = esb[h].reshape(P, 1)
    evB = esb[4 + h].reshape(P, 1)
    idm = np.eye(P, dtype=ml_dtypes.bfloat16)
    return m32, mwA, mwB, evA, evB, idm


def _in_maps(inputs):
    x = np.asarray(inputs["x"], dtype=np.float32)
    W_dist = np.asarray(inputs["W_dist"], dtype=np.float32)
    b_dist = np.asarray(inputs["b_dist"], dtype=np.float32).reshape(H, 1)
    W_merge = np.asarray(inputs["W_merge"], dtype=np.float32).reshape(1, H)
    b_merge = np.asarray(inputs["b_merge"], dtype=np.float32).reshape(1, 1)
    selfbias = np.asarray(inputs["selfbias"], dtype=np.float32).reshape(1, H)

    wst = np.ascontiguousarray(W_dist[:, :F].T).astype(ml_dtypes.bfloat16)
    wtt = np.ascontiguousarray(W_dist[:, F:].T).astype(ml_dtypes.bfloat16)
    m32, mwA, mwB, evA, evB, idm = _host_consts(W_merge, selfbias)

    in_maps = []
    for c in range(NCORES):
        xr = np.ascontiguousarray(np.roll(x, -c * ROWS, axis=0)).astype(
            ml_dtypes.bfloat16
        )
        in_maps.append(
            {
                "x": xr,
                "wst": wst,
                "wtt": wtt,
                "bd": b_dist,
                "bm": b_merge,
                "m32": m32,
                "mwA": mwA,
                "mwB": mwB,
                "evA": evA,
                "evB": evB,
                "idm": idm,
            }
        )
    return in_maps


def _assemble(results):
    out = np.empty((N, N), dtype=np.float32)
    for c in range(NCORES):
        out[c * ROWS : (c + 1) * ROWS, :] = np.roll(
            results[c]["out"], c * ROWS, axis=1
        )
    return out


def kernel(x, W_dist, b_dist, W_merge, b_merge, selfbias):
    nc = _get_nc()
    in_maps = _in_maps(
        {
            "x": x,
            "W_dist": W_dist,
            "b_dist": b_dist,
            "W_merge": W_merge,
            "b_merge": b_merge,
            "selfbias": selfbias,
        }
    )
    res = run_bass_kernel_spmd(nc, in_maps, core_ids=list(range(NCORES)))
    return _assemble(res.results)
